# revision 1
# baseline (speedup 1.0000x reference)
"""Multi-head attention (B=4, L=2048, D=768, H=12) on 8 Trainium2 NeuronCores.

Sharding: (batch, head-group). Core c handles batch c//2 and heads
6*(c%2) .. 6*(c%2)+6.  Each core computes its 6 heads' attention output and
the partial output projection y_part = AO @ Wo[rows]; the host sums the two
partials per batch, rescales, and adds biases.  No collectives.

Per-core pipeline (bf16 matmul operands; fp32 PSUM accumulate):
  A. QT = Wq_s^T@xT + bq, KT = Wk_s^T@xT  [384,2048] bf16 (feature-major)
     V  = x@Wv_s [2048, 6, 64+1] bf16 with a 1/16 ones column (denominator).
     bk is dropped (softmax shift-invariant); bv/bo folded on the host.
  B. per head: S^T[k,q] = K_h @ Q_h^T -> PSUM [128, 2048]; softmax numerator
     p = exp(s/8 - 2.5) in bf16, computed on ACT (real exp) for 9/16 of
     key-chunks and via the Schraudolph bit-trick on DVE for the rest:
     int32(s*A + B) makes the float bits directly, and the high 16 bits of
     each int32 ARE the (truncated) bf16 -- PV reads them through a strided
     bitcast view, so no second conversion op is needed.  PV runs
     query-major: out[q, 0:65] accumulated over key chunks; column 64 is
     the softmax denominator/16.  Eviction multiplies by the per-partition
     (per-query) reciprocal -> AO = 16*attn in bf16, query-major.
  C. AO is transposed to feature-major with the XBAR DMA transpose (one
     call per query chunk), then y = AO^T @ Wo in bf16, evicted to fp32
     and DMA'd out.  Host: (y0+y1)/16 + bv@Wo + bo.
"""

import numpy as np
import ml_dtypes

import concourse.mybir as mybir
import concourse.tile as tile
from concourse import bacc
from concourse.bass_utils import run_bass_kernel_spmd

F32 = mybir.dt.float32
BF16 = mybir.dt.bfloat16
I16 = mybir.dt.int16
P = 128
B, L, D, H = 4, 2048, 768, 12
HD = 64                    # head dim
HL = H // 2                # heads per core = 6
HO = HL * HD               # local feature dim = 384
KC = D // P                # contraction chunks over D = 6
LC = L // P                # L chunks (key chunks) = 16
MC = HO // P               # feature chunks = 3
QC = LC                    # query chunks = 16

# Schraudolph exp: exp(s/8 - 2.5) ~= bitcast_f32(int32(s*A + B))
LOG2E = 1.4426950408889634
SHIFT = 2.5
SCH_A = float(np.float32(0.125 * LOG2E * (1 << 7)))
SCH_B = float(np.float32((127.0 - SHIFT * LOG2E) * (1 << 7) - 365056.0 / (1 << 16)))

# exp flavor per (key-chunk, query-quarter): strict ACT/DVE alternation
# at quarter-block granularity so the two exp engines always overlap and
# the scores-psum reuse window (5 single-bank tiles) has real slack
def use_schraudolph(lk, q4):
    return (lk + q4) % 2 == 0

PV_LAG = 6   # PV trails scores by this many quarter-units

_NC = None


def s512(i):
    return slice(i * 512, (i + 1) * 512)


def build():
    nc = bacc.Bacc("TRN2", target_bir_lowering=False, debug=False)

    xT = nc.dram_tensor("xT", [D, L], BF16, kind="ExternalInput")
    # weights host-preswizzled: one DMA each, 128 partition-major rows
    wq = nc.dram_tensor("wq", [P, KC * HO], BF16, kind="ExternalInput")
    wk = nc.dram_tensor("wk", [P, KC * HO], BF16, kind="ExternalInput")
    wv = nc.dram_tensor("wv", [P, KC * HO], BF16, kind="ExternalInput")
    wo = nc.dram_tensor("wo", [P, MC * D], BF16, kind="ExternalInput")
    bq = nc.dram_tensor("bq", [HO], F32, kind="ExternalInput")
    y = nc.dram_tensor("y", [L, D], BF16, kind="ExternalOutput")

    with tile.TileContext(nc) as tc:
        with tc.tile_pool(name="static", bufs=1) as static:
            qT_tiles = [static.tile([P, L], BF16, name=f"qT{m}") for m in range(MC)]
            kT_tiles = [static.tile([P, L], BF16, name=f"kT{m}") for m in range(MC)]
            v_sb = static.tile([P, LC, HL, HD], BF16)
            ones_sb = static.tile([P, 1], BF16)
            ao_q = static.tile([P, QC, HL, HD], BF16)     # query-major AO
            ao_t = static.tile([P, QC, MC, P], BF16)      # feature-major AO
            bq_sb = static.tile([P, MC], F32)
            dummy_sb = static.tile([P, 1], F32)
            shift_sb = static.tile([P, 1], F32)

            # denominator ones vector = 1/16 (exact in bf16)
            nc.vector.memset(ones_sb[:], 0.0625)
            nc.vector.memset(shift_sb[:], -SHIFT)
            # preload the exp activation table while DMAs run
            nc.scalar.activation(
                out=dummy_sb[:],
                in_=shift_sb[:],
                func=mybir.ActivationFunctionType.Exp,
            )

            with (
                tc.tile_pool(name="xpool", bufs=1) as xpool,
                tc.tile_pool(name="wpool", bufs=1) as wpool,
            ):
                # DMA order: first QK matmul needs only wq chunk 0 + xT chunk 0
                xT_chunks = [xpool.tile([P, L], BF16, name=f"xc{c}") for c in range(KC)]
                HL2 = L // 2
                nc.sync.dma_start(xT_chunks[0][:, 0:HL2], xT[0:P, 0:HL2])
                wq_sb = wpool.tile([P, KC, HO], BF16, name="wq")
                wq_r = wq.ap().rearrange("p (c h) -> p c h", c=KC)
                for c in range(KC):
                    nc.sync.dma_start(wq_sb[:, c, :], wq_r[:, c, :])
                nc.sync.dma_start(xT_chunks[1][:, 0:HL2], xT[P : 2 * P, 0:HL2])
                wk_sb = wpool.tile([P, KC, HO], BF16, name="wk")
                nc.sync.dma_start(wk_sb[:, :, :], wk.ap().rearrange("p (c h) -> p c h", c=KC))
                nc.sync.dma_start(xT_chunks[2][:, 0:HL2], xT[2 * P : 3 * P, 0:HL2])
                wv_sb = wpool.tile([P, KC, HO], BF16, name="wv")
                nc.sync.dma_start(wv_sb[:, :, :], wv.ap().rearrange("p (c h) -> p c h", c=KC))
                for c in range(3, KC):
                    nc.sync.dma_start(xT_chunks[c][:, 0:HL2], xT[c * P : (c + 1) * P, 0:HL2])
                for c in range(KC):
                    nc.sync.dma_start(
                        xT_chunks[c][:, HL2:L], xT[c * P : (c + 1) * P, HL2:L]
                    )
                nc.sync.dma_start(bq_sb[:], bq.ap().rearrange("(c p) -> p c", p=P))
                wo_sb = wpool.tile([P, MC, D], BF16, name="wo")
                nc.sync.dma_start(
                    wo_sb[:, :, :], wo.ap().rearrange("p (c d) -> p c d", c=MC)
                )
                w_tiles = {"q": wq_sb, "k": wk_sb}

                # ------ single stream: projections woven into attention ------
                with (
                    tc.tile_pool(name="ppool", bufs=PV_LAG + 2) as ppool,
                    tc.tile_pool(name="ipool", bufs=PV_LAG + 2) as ipool,
                    tc.tile_pool(name="rpool", bufs=4) as rpool,
                    tc.tile_pool(name="sps", bufs=5, space="PSUM") as sps,
                    tc.tile_pool(name="pvps", bufs=2, space="PSUM") as pvps,
                    tc.tile_pool(name="dnps", bufs=1, space="PSUM") as dnps,
                ):
                    # one bank holds both head-parities' denominator groups
                    dn_t = dnps.tile([P, 2, QC], F32, name="dn")
                    nc.vector.memset(dn_t[:, :, :], 0.0)
                    def do_qk(i, n2):
                        # one 512-wide slice of a projection job
                        m, which, h = i // 4, (i // 2) % 2, i % 2
                        w_sb = w_tiles["q" if which == 0 else "k"]
                        ps = sps.tile([P, 512], F32, tag="s", name=f"qk{i}_{n2}")
                        j = h * 2 + n2
                        for k in range(KC):
                            nc.tensor.matmul(
                                ps[:, :],
                                w_sb[:, k, m * P : (m + 1) * P],
                                xT_chunks[k][:, s512(j)],
                                start=(k == 0),
                                stop=(k == KC - 1),
                            )
                        out_sb = (qT_tiles if which == 0 else kT_tiles)[m]
                        out_ap = out_sb[:, j * 512 : (j + 1) * 512]
                        if which == 0:
                            nc.scalar.activation(
                                out=out_ap,
                                in_=ps[:, :],
                                func=mybir.ActivationFunctionType.Identity,
                                bias=bq_sb[:, m : m + 1],
                            )
                        else:
                            nc.scalar.activation(
                                out=out_ap,
                                in_=ps[:, :],
                                func=mybir.ActivationFunctionType.Copy,
                            )

                    def do_v(l):
                        ps = sps.tile([P, 512], F32, tag="s", name=f"vj{l}")
                        for k in range(KC):
                            nc.tensor.matmul(
                                ps[:, 0:HO],
                                xT_chunks[k][:, l * P : (l + 1) * P],
                                wv_sb[:, k, :],
                                start=(k == 0),
                                stop=(k == KC - 1),
                            )
                        nc.vector.tensor_copy(
                            v_sb[:, l, :, :],
                            ps[:, 0:HO].rearrange("p (h d) -> p h d", d=HD),
                        )

                    def emit_scores_exp(hl, lk, q4):
                        pc, odd = hl // 2, hl % 2
                        r0 = odd * HD
                        sch = use_schraudolph(lk, q4)
                        s_t = sps.tile(
                            [P, 512], F32, tag="s", name=f"s{hl}_{lk}_{q4}"
                        )
                        nc.tensor.matmul(
                            s_t[:, :],
                            kT_tiles[pc][r0 : r0 + HD, lk * P : (lk + 1) * P],
                            qT_tiles[pc][r0 : r0 + HD, s512(q4)],
                            start=True,
                            stop=True,
                        )
                        if sch:
                            i_t = ipool.tile(
                                [P, 512], I16, tag="i", name=f"i{hl}_{lk}_{q4}"
                            )
                            nc.vector.tensor_scalar(
                                i_t[:, :],
                                s_t[:, :],
                                SCH_A,
                                SCH_B,
                                mybir.AluOpType.mult,
                                mybir.AluOpType.add,
                            )
                            # the int16 IS the truncated-bf16 bit pattern
                            p_ap = i_t[:, :].bitcast(BF16)
                        else:
                            p_t = ppool.tile(
                                [P, 512], BF16, tag="p", name=f"p{hl}_{lk}_{q4}"
                            )
                            nc.scalar.activation(
                                out=p_t[:, :],
                                in_=s_t[:, :],
                                func=mybir.ActivationFunctionType.Exp,
                                bias=shift_sb[:, 0:1],
                                scale=0.125,
                            )
                            p_ap = p_t[:, :]
                        return p_ap

                    pv_of = {}
                    evictq = []   # deferred per-qc eviction thunks

                    def emit_pv(hl, lk, q4, p_ap):
                        first, last = lk == 0, lk == LC - 1
                        half = q4 // 2
                        if (hl, half) not in pv_of:
                            pv_of[(hl, half)] = pvps.tile(
                                [P, 8, HD], F32, tag="pv", name=f"pv{hl}_{half}"
                            )
                        pv = pv_of[(hl, half)]
                        for j in range(4):
                            qc = q4 * 4 + j
                            # exactly one start per pv bank: it marks the
                            # whole bank pending-zero; later groups overwrite
                            # their own region on first touch, then accumulate
                            nc.tensor.matmul(
                                pv[:, qc % 8, :],
                                p_ap[:, j * P : (j + 1) * P],
                                v_sb[:, lk, hl, :],
                                start=(first and q4 % 2 == 0 and j == 0),
                                stop=last,
                                skip_group_check=True,
                            )
                        for j in range(4):
                            qc = q4 * 4 + j
                            # dn bank is never started: it is memset to zero
                            # before reuse, and matmuls always accumulate
                            nc.tensor.matmul(
                                dn_t[:, hl % 2, qc : qc + 1],
                                p_ap[:, j * P : (j + 1) * P],
                                ones_sb[:, :],
                                start=False,
                                stop=last,
                                skip_group_check=True,
                            )
                        if last and q4 == 1:
                            pv_of[(hl, "lo")] = pv_of.pop((hl, 0))
                        elif last and q4 == 3:
                            pv_hi = pv_of.pop((hl, 1))
                            pv_lo = pv_of.pop((hl, "lo"))
                            rstage = rpool.tile([P, QC], F32, tag="r")
                            nc.vector.reciprocal(rstage[:, :], dn_t[:, hl % 2, :])
                            # re-zero this parity's denominators for head hl+2
                            if hl + 2 < HL:
                                nc.vector.memset(dn_t[:, hl % 2, :], 0.0)

                            def evict(qc, hl=hl, pv_lo=pv_lo, pv_hi=pv_hi,
                                      rstage=rstage):
                                pv = pv_lo if qc < 8 else pv_hi
                                if qc % 2 == 0:
                                    nc.scalar.activation(
                                        out=ao_q[:, qc, hl, :],
                                        in_=pv[:, qc % 8, :],
                                        func=mybir.ActivationFunctionType.Copy,
                                        scale=rstage[:, qc : qc + 1],
                                    )
                                else:
                                    nc.vector.tensor_scalar_mul(
                                        ao_q[:, qc, hl, :],
                                        pv[:, qc % 8, :],
                                        rstage[:, qc : qc + 1],
                                    )

                            for qc in range(QC):
                                evictq.append((hl, qc, evict))

                    # Quarter-unit stream with PV lagging PV_LAG units so
                    # PE never waits on a unit's exp; projection jobs are
                    # woven between units (shared sps psum pool):
                    #   - v-jobs per key-chunk, just ahead of their first use
                    #   - m1/m2 qk-job slices inside heads 1 / 2-3
                    prejobs = {}   # quarter-slot index -> list of thunks
                    for i in range(4):
                        for n2 in range(2):
                            prejobs.setdefault(4 * (17 + 3 * i) + 2 * n2, []).append(
                                lambda i=i, n2=n2: do_qk(4 + i, n2)
                            )
                            prejobs.setdefault(4 * (38 + 6 * i) + 2 * n2, []).append(
                                lambda i=i, n2=n2: do_qk(8 + i, n2)
                            )
                    for l in range(2, LC):
                        prejobs.setdefault(4 * (l - 2), []).append(lambda l=l: do_v(l))

                    # prelude: qT[0]/kT[0] + first two v chunks; h=0
                    # slices first (they need only the first xT halves)
                    for i in (0, 2, 1, 3):
                        do_qk(i, 0)
                        do_qk(i, 1)
                    do_v(0)
                    do_v(1)

                    slots = [
                        (hl, lk, q4)
                        for hl in range(HL)
                        for lk in range(LC)
                        for q4 in range(4)
                    ]
                    pend = []
                    for si, (hl, lk, q4) in enumerate(slots):
                        for job in prejobs.get(si, ()):
                            job()
                        p_ap = emit_scores_exp(hl, lk, q4)
                        pend.append((hl, lk, q4, p_ap))
                        # adaptive lag: drain hard at the head's end so the
                        # final PV units + evictions land before the next
                        # head needs the pv/dn buffers back
                        lag = PV_LAG if lk < LC - 1 else 3
                        while len(pend) > lag:
                            emit_pv(*pend.pop(0))
                        # spread pending evictions: a few per quarter-slot
                        for _ in range(3):
                            if evictq:
                                ehl, eqc, ethunk = evictq.pop(0)
                                ethunk(eqc)
                    for job in pend:
                        emit_pv(*job)
                    # tail: last head's evictions interleaved with the XBAR
                    # transposes so output projection can start immediately
                    done_t = set()

                    def quad_transpose(g):
                        nc.sync.dma_start_transpose(
                            ao_t[:, 4 * g : 4 * g + 4, :, :],
                            ao_q[:, 4 * g : 4 * g + 4, :, :],
                        )
                        done_t.add(g)

                    evicted = set()
                    for ehl, eqc, ethunk in evictq:
                        ethunk(eqc)
                        if ehl == HL - 1:
                            evicted.add(eqc)
                            g = eqc // 4
                            if all(4 * g + j in evicted for j in range(4)):
                                quad_transpose(g)
                    evictq.clear()
                    for g in range(QC // 4):
                        if g not in done_t:
                            quad_transpose(g)

                # ---------------- output projection ----------------
                with (
                    tc.tile_pool(name="ypool", bufs=4) as ypool,
                    tc.tile_pool(name="yps", bufs=3, space="PSUM") as yps,
                ):
                    for m in range(QC):
                        ps = yps.tile([P, D], F32, tag="y")
                        for c in range(MC):
                            for n0, nsz in ((0, 512), (512, 256)):
                                nc.tensor.matmul(
                                    ps[:, n0 : n0 + nsz],
                                    ao_t[:, m, c, :],
                                    wo_sb[:, c, n0 : n0 + nsz],
                                    start=(c == 0),
                                    stop=(c == MC - 1),
                                )
                        y_t = ypool.tile([P, D], BF16, tag="yt")
                        if m % 2 == 0:
                            nc.vector.tensor_copy(y_t[:], ps[:])
                        else:
                            nc.scalar.activation(
                                out=y_t[:],
                                in_=ps[:],
                                func=mybir.ActivationFunctionType.Copy,
                            )
                        yeng = nc.sync if m % 2 == 0 else nc.scalar
                        yeng.dma_start(y[m * P : (m + 1) * P, :], y_t[:])

    nc.compile()
    return nc


def _get_nc():
    global _NC
    if _NC is None:
        _NC = build()
    return _NC


def _swizzle_w(Wslice):
    # [D, N] -> [128, KCxN]: row p holds chunks c of rows c*128+p
    Dd, N = Wslice.shape
    c = Dd // P
    return np.ascontiguousarray(
        Wslice.reshape(c, P, N).transpose(1, 0, 2).reshape(P, c * N)
    )


def kernel(**inputs) -> np.ndarray:
    x = np.asarray(inputs["x"], dtype=np.float32)
    Wq = np.asarray(inputs["Wq"], dtype=np.float32)
    Wk = np.asarray(inputs["Wk"], dtype=np.float32)
    Wv = np.asarray(inputs["Wv"], dtype=np.float32)
    Wo = np.asarray(inputs["Wo"], dtype=np.float32)
    bq = np.asarray(inputs["bq"], dtype=np.float32)
    bv = np.asarray(inputs["bv"], dtype=np.float32)
    bo = np.asarray(inputs["bo"], dtype=np.float32)

    nc = _get_nc()
    bf = ml_dtypes.bfloat16

    in_maps = []
    for c in range(8):
        b, hg = c // 2, c % 2
        cs = slice(hg * HO, (hg + 1) * HO)
        in_maps.append(
            {
                "xT": np.ascontiguousarray(x[b].T).astype(bf),
                "wq": _swizzle_w(Wq[:, cs]).astype(bf),
                "wk": _swizzle_w(Wk[:, cs]).astype(bf),
                "wv": _swizzle_w(Wv[:, cs]).astype(bf),
                "wo": _swizzle_w(Wo[cs, :]).astype(bf),
                "bq": np.ascontiguousarray(bq[cs]),
            }
        )

    res = run_bass_kernel_spmd(nc, in_maps, core_ids=list(range(8)))
    bias_full = bv @ Wo + bo
    out = np.empty((B, L, D), dtype=np.float32)
    for b in range(B):
        out[b] = (
            res.results[2 * b]["y"].astype(np.float32)
            + res.results[2 * b + 1]["y"].astype(np.float32)
        ) / 16.0 + bias_full
    return out

